# revision 25
# baseline (speedup 1.0000x reference)
"""Trainium2 Bass kernel for nn_EquivariantBiLinear.

Math (per batch row b):
    pieces:  Y[k, b] = sum_nu W_g[mu, nu] * x[b, bid_g[nu*r+rho]]   (k = off_g + mu*r + rho)
    out[b, o] = 0.1 * sum_i Y[W_invperm[o*256+i], b] * x[b, i]

Sharding: 4-way over batch x 2-way over k-space (8 cores). Each core
handles 512 batch rows and half the 65536 k-rows; the host adds the two
k-half partials per batch slice (outside HW exec). N=512 streams make
every matmul stream-bound instead of weight-load-bound.

Per core the group GEMMs run in fp16 and produce Y in (128 k, 512 b)
chunks. The random permutation k -> (o, i) is applied with one-hot
matmuls whose lhsT is stored fp8 (exact for 0/1, half the DMA) against
fp16 data rhs (PE streams 1 col/cycle regardless of dtype, so fp8
DoubleRow buys nothing at fixed column count):
  - gather: xtg = RT8 @ xts16 (2 mms per chunk over i-halves).
  - z = y * xtg on DVE (fp16).
  - scatter: outT[H] += OH8_H @ z per chunk, one PSUM bank per o-half
    H -- sharing a bank breaks because the start bit pending-zeroes
    the whole 2KB bank.
GEMM rhs tiles (x-column gathers) are pre-gathered on host and streamed
as one dense fp16 tensor. The 0.1 scale and the /4 of xts fold into a
0.4 epilogue scale.
"""

import sys

if "/opt/trn_rl_repo" not in sys.path:
    sys.path.insert(0, "/opt/trn_rl_repo")

from contextlib import ExitStack

import numpy as np
import ml_dtypes

import concourse.bacc as bacc
import concourse.mybir as mybir
import concourse.tile as tile
from concourse.bass_utils import run_bass_kernel_spmd
from concourse.masks import make_identity

GROUPS = [(512, 1, 16384), (256, 4, 4096), (128, 16, 1024), (64, 64, 256)]
X = 256
B = 2048
NCORES = 8
BS = 512  # batch rows per core (4 slices x 2 k-halves)
NCHUNK = 256  # chunks per core (k-half)
NPAIR = NCHUNK // 2

F32 = mybir.dt.float32
FP16 = mybir.dt.float16
FP8 = mybir.dt.float8e4
DR = mybir.MatmulPerfMode.DoubleRow
E4M3 = ml_dtypes.float8_e4m3


def _chunk_klists(kh):
    """Global k indices (128 per chunk) in device production order for
    k-half kh. Blocks of 2 chunks share one PSUM ps tile."""
    ks = []
    p = np.arange(128)
    for mp in range(8):
        for mq2 in range(4):
            for sub2 in range(2):
                mt = (kh * 8 + mp) * 8 + mq2 * 2 + sub2
                ks.append(mt * 128 + p)
    for mt in range(16):
        mt_g = kh * 16 + mt
        for j in range(4):
            ks.append(16384 + (mt_g * 128 + p) * 4 + j)
    for mt in range(4):
        mt_g = kh * 4 + mt
        for np2 in range(4):
            for j in range(4):
                ks.append(32768 + (mt_g * 128 + p) * 16 + np2 * 4 + j)
    for s in range(2):
        for np3 in range(8):
            for j in range(4):
                ks.append(49152 + (kh * 128 + p) * 64 + 2 * (np3 * 4 + j) + s)
    assert len(ks) == NCHUNK
    return ks


def _host_prep(W0, W1, W2, W3, bid0, bid1, bid2, bid3, W_invperm):
    """Pure layout transforms of weights/indices (no arithmetic on data)."""
    Ws = [np.asarray(W) for W in (W0, W1, W2, W3)]
    bids = [np.asarray(b).astype(np.int64) for b in (bid0, bid1, bid2, bid3)]
    wt = []
    for (n, r, m), W in zip(GROUPS, Ws):
        wt.append(np.ascontiguousarray(W.reshape(m, n).T.astype(np.float16)))
    wt3 = np.concatenate([wt[3], wt[3]], axis=0)  # (128, 256)

    # per-k-half weight slices
    wts = []
    for kh in range(2):
        wts.append(
            (
                np.ascontiguousarray(wt[0][:, kh * 8192 : (kh + 1) * 8192]),
                np.ascontiguousarray(wt[1][:, kh * 2048 : (kh + 1) * 2048]),
                np.ascontiguousarray(wt[2][:, kh * 512 : (kh + 1) * 512]),
                np.ascontiguousarray(wt3[:, kh * 128 : (kh + 1) * 128]),
            )
        )

    # x-gather column map (128, 60): which x column feeds xrep[nu, tcol]
    cols = []
    b0 = bids[0]
    for kc in range(4):
        cols.append(b0[kc * 128 : (kc + 1) * 128])
    b1 = bids[1].reshape(256, 4)
    for kc in range(2):
        for rho in range(4):
            cols.append(b1[kc * 128 : (kc + 1) * 128, rho])
    b2 = bids[2].reshape(128, 16)
    for rho in range(16):
        cols.append(b2[:, rho])
    b3 = bids[3].reshape(64, 64)
    p = np.arange(128)
    for q in range(32):
        cols.append(b3[p % 64, 2 * q + p // 64])
    xgidx = np.ascontiguousarray(np.stack(cols, axis=1).astype(np.int64))  # (128, 60)

    # inverse of W_invperm: perm[k] = o*256 + i position of Y row k
    ivp = np.asarray(W_invperm).astype(np.int64)
    perm = np.empty(X * X, np.int64)
    perm[ivp] = np.arange(X * X)

    pr = np.arange(128)
    rt8s, oh8s = [], []
    for kh in range(2):
        klists = _chunk_klists(kh)
        # rt8[i', c*256 + h*128 + p] = [i_k(c,p) == h*128 + i']
        rt8 = np.zeros((128, NCHUNK * 256), E4M3)
        # oh8[p, c*256 + H*128 + o'] = [o_k(c,p) == H*128 + o']
        oh8 = np.zeros((128, NCHUNK * 256), E4M3)
        for c, kl in enumerate(klists):
            pk = perm[kl]
            ik = pk % X
            ok = pk // X
            rt8[ik % 128, c * 256 + (ik // 128) * 128 + pr] = 1.0
            oh8[pr, c * 256 + (ok // 128) * 128 + (ok % 128)] = 1.0
        rt8s.append(rt8)
        oh8s.append(oh8)
    return wts, xgidx, rt8s, oh8s


def _build_nc():
    nc = bacc.Bacc("TRN2", target_bir_lowering=False, debug=False, num_devices=NCORES)

    xrep_d = nc.dram_tensor("xrep", [128, 60 * BS], FP16, kind="ExternalInput")
    xts8_d = nc.dram_tensor("xts8", [128, 2 * BS], FP16, kind="ExternalInput")
    rt8_d = nc.dram_tensor("rt8", [128, NCHUNK * 256], FP8, kind="ExternalInput")
    oh8_d = nc.dram_tensor("oh8", [128, NCHUNK * 256], FP8, kind="ExternalInput")
    wt_d = [
        nc.dram_tensor("wt0", [512, 8192], FP16, kind="ExternalInput"),
        nc.dram_tensor("wt1", [256, 2048], FP16, kind="ExternalInput"),
        nc.dram_tensor("wt2", [128, 512], FP16, kind="ExternalInput"),
        nc.dram_tensor("wt3", [128, 128], FP16, kind="ExternalInput"),
    ]
    out_d = nc.dram_tensor("out", [BS, X], F32, kind="ExternalOutput")

    with tile.TileContext(nc) as tc, ExitStack() as ctx:
        const = ctx.enter_context(tc.tile_pool(name="const", bufs=1))
        wpool = ctx.enter_context(tc.tile_pool(name="wpool", bufs=4))
        rtpool = ctx.enter_context(tc.tile_pool(name="rtpool", bufs=3))
        ohpool = ctx.enter_context(tc.tile_pool(name="ohpool", bufs=3))
        ypool = ctx.enter_context(tc.tile_pool(name="ypool", bufs=6))
        zpool = ctx.enter_context(tc.tile_pool(name="zpool", bufs=6))
        pgemm = ctx.enter_context(tc.tile_pool(name="pgemm", bufs=2, space="PSUM"))
        pxtg = ctx.enter_context(tc.tile_pool(name="pxtg", bufs=1, space="PSUM"))
        pout = ctx.enter_context(tc.tile_pool(name="pout", bufs=1, space="PSUM"))

        ident = const.tile([128, 128], F32)
        make_identity(nc, ident[:])

        # per-group xrep tiles so g0 only waits on its own 2MB
        xrep0t = const.tile([128, 4 * BS], FP16)
        nc.sync.dma_start(xrep0t[:], xrep_d[:, 0 : 4 * BS])
        xts16 = const.tile([128, 2 * BS], FP16)
        nc.sync.dma_start(xts16[:], xts8_d[:])
        xrep1t = const.tile([128, 8 * BS], FP16)
        nc.scalar.dma_start(xrep1t[:], xrep_d[:, 4 * BS : 12 * BS])
        xrep2t = const.tile([128, 16 * BS], FP16)
        nc.scalar.dma_start(xrep2t[:], xrep_d[:, 12 * BS : 28 * BS])
        xrep3t = const.tile([128, 32 * BS], FP16)
        nc.scalar.dma_start(xrep3t[:], xrep_d[:, 28 * BS : 60 * BS])
        w1t = const.tile([128, 4096], FP16)
        nc.scalar.dma_start(w1t[:, 0:2048], wt_d[1][0:128, :])
        nc.scalar.dma_start(w1t[:, 2048:4096], wt_d[1][128:256, :])
        w2t = const.tile([128, 512], FP16)
        nc.scalar.dma_start(w2t[:], wt_d[2][:])
        w3t = const.tile([128, 128], FP16)
        nc.scalar.dma_start(w3t[:], wt_d[3][:])

        xrep0 = [xrep0t[:, kc * BS : (kc + 1) * BS] for kc in range(4)]
        xrep1 = [xrep1t[:, 4 * kc * BS : 4 * (kc + 1) * BS] for kc in range(2)]
        xrep2 = xrep2t[:]
        xrep3 = xrep3t[:]

        # persistent output accumulators: one PSUM BANK per o-half
        outT_ps = [
            pout.tile([128, BS], F32, tag=f"pout{h}", name=f"pout{h}") for h in range(2)
        ]

        state = {"c": 0, "rtt": None, "oht": None, "pend": [], "rtq": [], "ohq": []}

        def fetch_rt(w):
            if w * 8 < NCHUNK:
                rtt = rtpool.tile([128, 2048], FP8, tag="rtt", name="rtt")
                nc.sync.dma_start(rtt[:], rt8_d[:, w * 8 * 256 : (w + 1) * 8 * 256])
                state["rtq"].append(rtt)

        def fetch_oh(w):
            if w * 16 < NCHUNK:
                oht = ohpool.tile([128, 4096], FP8, tag="oht", name="oht")
                nc.sync.dma_start(oht[:], oh8_d[:, w * 16 * 256 : (w + 1) * 16 * 256])
                state["ohq"].append(oht)

        fetch_rt(0)
        fetch_oh(0)
        fetch_rt(1)
        fetch_oh(1)

        def flush_pending():
            for c, ohs, zv in state["pend"]:
                for H in range(2):
                    nc.tensor.matmul(
                        outT_ps[H][:],
                        ohs[:, H * 128 : (H + 1) * 128],
                        zv,
                        start=(c == 0),
                        stop=(c == NCHUNK - 1),
                        skip_group_check=True,
                    )
            state["pend"].clear()

        def fuse_block(yt):
            """Consume one block: 2 chunks of Y (fp16 [128, 2*512])."""
            c0 = state["c"]
            state["c"] = c0 + 2
            if len(state["pend"]) >= 8:
                flush_pending()
            if c0 % 8 == 0:
                state["rtt"] = state["rtq"].pop(0)
                fetch_rt(c0 // 8 + 2)
            if c0 % 16 == 0:
                state["oht"] = state["ohq"].pop(0)
                fetch_oh(c0 // 16 + 2)
            xtg2 = pxtg.tile([128, 1024], F32, tag="xtg2", name="xtg2")
            for j in range(2):
                co = ((c0 + j) % 8) * 256
                for h in range(2):
                    nc.tensor.matmul(
                        xtg2[:, j * BS : (j + 1) * BS],
                        state["rtt"][:, co + h * 128 : co + (h + 1) * 128],
                        xts16[:, h * BS : (h + 1) * BS],
                        start=(h == 0),
                        stop=(h == 1),
                    )
            z16 = zpool.tile([128, 1024], FP16, tag="z16", name="z16")
            nc.vector.tensor_mul(z16[:], yt[:], xtg2[:])
            for j in range(2):
                c = c0 + j
                ohs = state["oht"][:, (c % 16) * 256 : (c % 16 + 1) * 256]
                state["pend"].append((c, ohs, z16[:, j * BS : (j + 1) * BS]))

        def psum_to_sbuf(ps_ap):
            yt = ypool.tile([128, 1024], FP16, tag="ytile", name="yt")
            nc.scalar.copy(yt[:], ps_ap)
            return yt

        # ---- group GEMMs (fp16, N=512) with fused consumption ----
        # g0: k = mu
        for mp in range(8):
            w0t = [
                wpool.tile([128, 1024], FP16, tag=f"w0_{kc}", name=f"w0_{kc}")
                for kc in range(4)
            ]
            for kc in range(4):
                nc.sync.dma_start(
                    w0t[kc][:],
                    wt_d[0][kc * 128 : (kc + 1) * 128, mp * 1024 : (mp + 1) * 1024],
                )
            for mq2 in range(4):
                ps = pgemm.tile([128, 1024], F32, tag="pg", name="ps")
                for sub2 in range(2):
                    mt = mq2 * 2 + sub2
                    for kc in range(4):
                        nc.tensor.matmul(
                            ps[:, sub2 * 512 : (sub2 + 1) * 512],
                            w0t[kc][:, mt * 128 : (mt + 1) * 128],
                            xrep0[kc],
                            start=(kc == 0),
                            stop=(kc == 3),
                        )
                fuse_block(psum_to_sbuf(ps[:]))

        # g1: k = 16384 + mu*4 + rho
        for mt in range(16):
            for jh in range(2):
                ps = pgemm.tile([128, 1024], F32, tag="pg", name="ps")
                for j2 in range(2):
                    j = jh * 2 + j2
                    for kc in range(2):
                        nc.tensor.matmul(
                            ps[:, j2 * 512 : (j2 + 1) * 512],
                            w1t[:, kc * 2048 + mt * 128 : kc * 2048 + (mt + 1) * 128],
                            xrep1[kc][:, j * BS : (j + 1) * BS],
                            start=(kc == 0),
                            stop=(kc == 1),
                        )
                fuse_block(psum_to_sbuf(ps[:]))

        # g2: k = 32768 + mu*16 + rho
        for mt in range(4):
            for np2 in range(4):
                for jh in range(2):
                    ps = pgemm.tile([128, 1024], F32, tag="pg", name="ps")
                    for j2 in range(2):
                        j = jh * 2 + j2
                        nc.tensor.matmul(
                            ps[:, j2 * 512 : (j2 + 1) * 512],
                            w2t[:, mt * 128 : (mt + 1) * 128],
                            xrep2[:, (np2 * 4 + j) * BS : (np2 * 4 + j + 1) * BS],
                            start=True,
                            stop=True,
                        )
                    fuse_block(psum_to_sbuf(ps[:]))

        # g3: k = 49152 + mu*64 + 2q + s
        for s in range(2):
            for np3 in range(8):
                for jh in range(2):
                    ps = pgemm.tile([128, 1024], F32, tag="pg", name="ps")
                    for j2 in range(2):
                        j = jh * 2 + j2
                        q = np3 * 4 + j
                        nc.tensor.matmul(
                            ps[:, j2 * 512 : (j2 + 1) * 512],
                            w3t[s * 64 : (s + 1) * 64, :],
                            xrep3[s * 64 : (s + 1) * 64, q * BS : (q + 1) * BS],
                            start=True,
                            stop=True,
                        )
                    fuse_block(psum_to_sbuf(ps[:]))

        assert state["c"] == NCHUNK
        flush_pending()

        # ---- epilogue: out[b, o] = 0.4 * outT[o, b] ----
        outstage = [
            const.tile([128, 256], F32, tag=f"outstage{bh}", name=f"outstage{bh}")
            for bh in range(4)
        ]
        for H in range(2):
            outT_sb = zpool.tile([128, BS], F32, tag="outT_sb", name="outT_sb", bufs=2)
            nc.vector.tensor_scalar_mul(outT_sb[:], outT_ps[H][:], 0.4)
            for bh in range(4):
                pst2 = pgemm.tile([128, 1024], F32, tag="pg", name="pst2")
                nc.tensor.transpose(
                    pst2[:, 0:128], outT_sb[:, bh * 128 : (bh + 1) * 128], ident[:]
                )
                nc.any.tensor_copy(
                    outstage[bh][:, H * 128 : (H + 1) * 128], pst2[:, 0:128]
                )
        for bh in range(4):
            nc.sync.dma_start(out_d[bh * 128 : (bh + 1) * 128, :], outstage[bh][:])

    nc.compile()
    return nc


_NC_CACHE = None


def _make_in_maps(x, wts, xgidx, rt8s, oh8s):
    x = np.ascontiguousarray(np.asarray(x, dtype=np.float32))
    in_maps = []
    for c in range(NCORES):
        bc, kh = divmod(c, 2)
        xsh = x[bc * BS : (bc + 1) * BS, :]
        xg = xsh[:, xgidx]  # (512 b, 128 nu, 60 t)
        xrep = np.ascontiguousarray(
            xg.transpose(1, 2, 0).reshape(128, 60 * BS).astype(np.float16)
        )
        v = xsh.T.astype(np.float32) / 4.0  # (256 i, 512 b)
        xts8 = np.empty((128, 2 * BS), np.float16)
        for h in range(2):
            xts8[:, h * BS : (h + 1) * BS] = v[h * 128 : (h + 1) * 128]
        wt0, wt1, wt2, wt3 = wts[kh]
        in_maps.append(
            {
                "xrep": xrep,
                "xts8": np.ascontiguousarray(xts8),
                "rt8": rt8s[kh],
                "oh8": oh8s[kh],
                "wt0": wt0,
                "wt1": wt1,
                "wt2": wt2,
                "wt3": wt3,
            }
        )
    return in_maps


def kernel(x, W0, W1, W2, W3, bid0, bid1, bid2, bid3, W_invperm, **_unused):
    global _NC_CACHE
    prep = _host_prep(W0, W1, W2, W3, bid0, bid1, bid2, bid3, W_invperm)
    if _NC_CACHE is None:
        _NC_CACHE = _build_nc()
    nc = _NC_CACHE

    in_maps = _make_in_maps(x, *prep)
    res = run_bass_kernel_spmd(nc, in_maps, core_ids=list(range(NCORES)))
    outs = [np.asarray(res.results[c]["out"], np.float32) for c in range(NCORES)]
    out = np.concatenate(
        [outs[2 * bc] + outs[2 * bc + 1] for bc in range(NCORES // 2)], axis=0
    )
    return out.astype(np.float32)
